# revision 12
# baseline (speedup 1.0000x reference)
"""DynamicChunker Trainium2 kernel.

Strategy
--------
The reference runs a sequential scan over chunk starts, but each scan step is a
pure function of the window start position s. We therefore evaluate ALL
B*T = 128 possible starts x K=16 candidate lengths in parallel on 8 NeuronCores
(16 starts per core), and do the trivial pointer-chasing chunk chain on host.

Per (start, length) only the cls row (query 0) of the tiny transformer matters:
  - scores for the cls query fold to one matvec:  s_h[k] = x_k . (SCALE*Wk_h^T q0_h) + c_h
  - prefix softmax over candidate lengths comes from cumulative sums of
    e = exp(s) and e*V along the window axis (all 16 lengths in one pass).
  - out-proj -> LN -> FF(2048) -> LN -> classifier evaluated for the cls row
    only, batched as 256 columns per core (16 starts x 16 lengths).

Layout: feature-major. SBUF tiles [128 partitions = feature chunk, 256 free
columns]. Columns are ordered col = half*128 + j*8 + p (j = token/length index,
p = problem-in-half) so each 128-column half is an independent problem set; the
attention + layernorm chains are emitted stage-interleaved over the two halves
to keep the PE fed while DVE/ACT serial chains run. Reciprocals are computed as
exp(-ln(x)) on the scalar engine (DVE InstReciprocal is ~1.7us per call).
"""

import numpy as np

import concourse.bass as bass
import concourse.bacc as bacc
import concourse.mybir as mybir
import concourse.tile as tile
from concourse.bass_utils import run_bass_kernel_spmd

F32 = mybir.dt.float32
AF = mybir.ActivationFunctionType
ALU = mybir.AluOpType

B, T, D, K, H, FFD = 2, 64, 256, 16, 4, 2048
HD = D // H
EPS = 1e-5
THR = 0.5
SCALE = 1.0 / float(np.sqrt(HD))
NCORES = 8
PPC = (B * T) // NCORES          # problems (starts) per core = 16
NCOL = PPC * K                   # 256 columns per core
NCH = D // 128                   # feature chunks = 2
NF = FFD // 128                  # ff chunks = 16
HW = 128                         # columns per half
PH = PPC // 2                    # problems per half = 8

# packed per-partition scalar columns in `scal` [128, 18]
S_BOCLS = 0    # bo + cls          (2 cols)
S_BV = 2       # bv                (2)
S_LN1W = 4     # (2)
S_LN1B = 6     # (2)
S_LN2W = 8     # (2)
S_LN2B = 10    # (2)
S_B2 = 12      # (2)
S_C1 = 14      # clf1_b            (2)
S_INIT = 16    # e0*v_cls          (2)
NSCAL = 18


def _build_nc(c2_const: float):
    nc = bacc.Bacc(None, target_bir_lowering=False, debug=False)

    xt_d = nc.declare_dram_parameter("xt", [128, NCH, NCOL], F32, isOutput=False)
    wtilde_d = nc.declare_dram_parameter("wtilde", [128, NCH, H], F32, isOutput=False)
    sm4_d = nc.declare_dram_parameter("sm4", [H, 2], F32, isOutput=False)
    hb_d = nc.declare_dram_parameter("hb", [H, NCH, 128], F32, isOutput=False)
    wv_d = nc.declare_dram_parameter("wv", [128, NCH, NCH, 128], F32, isOutput=False)
    wo_d = nc.declare_dram_parameter("wo", [128, NCH, NCH, 128], F32, isOutput=False)
    c1w_d = nc.declare_dram_parameter("c1w", [128, NCH, NCH, 128], F32, isOutput=False)
    w1_d = nc.declare_dram_parameter("w1", [128, NF, NCH, 128], F32, isOutput=False)
    w2_d = nc.declare_dram_parameter("w2", [128, NF, NCH, 128], F32, isOutput=False)
    scal_d = nc.declare_dram_parameter("scal", [128, NSCAL], F32, isOutput=False)
    b1c_d = nc.declare_dram_parameter("b1c", [128, NF], F32, isOutput=False)
    clf2_d = nc.declare_dram_parameter("clf2", [128, NCH], F32, isOutput=False)

    h2o_d = nc.declare_dram_parameter("h2o", [NCH, 128, NCOL], F32, isOutput=True)
    po_d = nc.declare_dram_parameter("po", [1, NCOL], F32, isOutput=True)

    HS = [slice(0, HW), slice(HW, NCOL)]     # half column slices

    with tile.TileContext(nc) as tc:
        with (
            tc.tile_pool(name="w", bufs=1) as wp,
            tc.tile_pool(name="act", bufs=1) as ap,
            tc.tile_pool(name="rp", bufs=4) as rp,
            tc.tile_pool(name="psA", bufs=3, space="PSUM") as psA,
            tc.tile_pool(name="psB", bufs=2, space="PSUM") as psB,
            tc.tile_pool(name="psC", bufs=3, space="PSUM") as psC,
        ):
            # ---- weight/const loads (issued in consumption order) ----
            xt = wp.tile([128, NCH, NCOL], F32, tag="xt")
            nc.sync.dma_start(xt[:], xt_d[:])
            wtilde = wp.tile([128, NCH, H], F32, tag="wtilde")
            nc.sync.dma_start(wtilde[:], wtilde_d[:])
            sm4 = wp.tile([H, 2], F32, tag="sm4")
            nc.sync.dma_start(sm4[:], sm4_d[:])
            hb = wp.tile([H, NCH, 128], F32, tag="hb")
            nc.sync.dma_start(hb[:], hb_d[:])
            wv = wp.tile([128, NCH, NCH, 128], F32, tag="wv")
            nc.sync.dma_start(wv[:], wv_d[:])
            scal = wp.tile([128, NSCAL], F32, tag="scal")
            nc.sync.dma_start(scal[:], scal_d[:])
            wo = wp.tile([128, NCH, NCH, 128], F32, tag="wo")
            nc.sync.dma_start(wo[:], wo_d[:])
            w1 = wp.tile([128, NF, NCH, 128], F32, tag="w1")
            for fg in range(4):
                nc.sync.dma_start(w1[:, 4 * fg:4 * fg + 4, :, :],
                                  w1_d[:, 4 * fg:4 * fg + 4, :, :])
            w2 = wp.tile([128, NF, NCH, 128], F32, tag="w2")
            for fg in range(4):
                nc.sync.dma_start(w2[:, 4 * fg:4 * fg + 4, :, :],
                                  w2_d[:, 4 * fg:4 * fg + 4, :, :])
            b1c = wp.tile([128, NF], F32, tag="b1c")
            nc.sync.dma_start(b1c[:], b1c_d[:])
            c1w = wp.tile([128, NCH, NCH, 128], F32, tag="c1w")
            nc.sync.dma_start(c1w[:], c1w_d[:])
            clf2 = wp.tile([128, NCH], F32, tag="clf2")
            nc.sync.dma_start(clf2[:], clf2_d[:])

            ones = wp.tile([128, 1], F32, tag="ones")
            nc.gpsimd.memset(ones[:], 1.0)
            ones1 = wp.tile([1, 128], F32, tag="ones1")
            nc.gpsimd.memset(ones1[:], 1.0)
            cst = wp.tile([1, 2], F32, tag="cst")
            nc.gpsimd.memset(cst[:, 0:1], EPS)
            nc.gpsimd.memset(cst[:, 1:2], float(c2_const))

            # shared full-width activation tiles (halves write their slices)
            e_sb = ap.tile([H, NCOL], F32, tag="e_sb")
            et = ap.tile([H, NCOL], F32, tag="et")
            rec4 = ap.tile([H, NCOL], F32, tag="rec4")
            v_sb = [ap.tile([128, NCOL], F32, tag=f"v{m}", name=f"v{m}")
                    for m in range(NCH)]
            ev_a = [ap.tile([128, NCOL], F32, tag=f"eva{m}", name=f"eva{m}")
                    for m in range(NCH)]
            ev_b = [ap.tile([128, NCOL], F32, tag=f"evb{m}", name=f"evb{m}")
                    for m in range(NCH)]
            o0 = [ap.tile([128, NCOL], F32, tag=f"o0{m}", name=f"o0{m}")
                  for m in range(NCH)]
            pre1sq = [ap.tile([128, 2, NCOL], F32, tag=f"p1s{m}", name=f"p1s{m}")
                      for m in range(NCH)]
            h1 = [ap.tile([128, NCOL], F32, tag=f"h1_{m}", name=f"h1_{m}")
                  for m in range(NCH)]
            bsrc1 = [ap.tile([1, 256], F32, tag=f"bs1_{h}", name=f"bs1_{h}")
                     for h in range(2)]
            bsrc2 = [ap.tile([1, 256], F32, tag=f"bs2_{h}", name=f"bs2_{h}")
                     for h in range(2)]
            var1 = [ap.tile([1, HW], F32, tag=f"var1_{h}", name=f"var1_{h}")
                    for h in range(2)]
            var2 = [ap.tile([1, HW], F32, tag=f"var2_{h}", name=f"var2_{h}")
                    for h in range(2)]
            pre2sq = [ap.tile([128, 2, NCOL], F32, tag=f"p2s{m}", name=f"p2s{m}")
                      for m in range(NCH)]
            h2 = [ap.tile([128, NCOL], F32, tag=f"h2_{m}", name=f"h2_{m}")
                  for m in range(NCH)]

            # ============ attention + LN1, two halves stage-interleaved ======
            # stage 1: scores + exp
            for h in range(2):
                hs = HS[h]
                ps_sc = psC.tile([H, HW], F32, tag="small", name=f"ps_sc{h}")
                for kc in range(NCH):
                    nc.tensor.matmul(ps_sc[:], wtilde[:, kc, :], xt[:, kc, hs],
                                     start=(kc == 0), stop=(kc == NCH - 1))
                nc.scalar.activation(e_sb[:, hs], ps_sc[:], AF.Exp,
                                     bias=sm4[:, 0:1])
            # stage 2: V projection
            for h in range(2):
                hs = HS[h]
                for mc in range(NCH):
                    ps_v = psA.tile([128, HW], F32, tag="t", name=f"ps_v{h}{mc}")
                    for kc in range(NCH):
                        nc.tensor.matmul(ps_v[:], wv[:, kc, mc, :], xt[:, kc, hs],
                                         start=(kc == 0), stop=(kc == NCH - 1))
                    nc.scalar.activation(v_sb[mc][:, hs], ps_v[:], AF.Identity,
                                         bias=scal[:, S_BV + mc:S_BV + mc + 1])
            # stage 3: e broadcast + ev
            for h in range(2):
                hs = HS[h]
                for mc in range(NCH):
                    ps_ef = psA.tile([128, HW], F32, tag="t", name=f"ps_ef{h}{mc}")
                    nc.tensor.matmul(ps_ef[:], hb[:, mc, :], e_sb[:, hs])
                    nc.vector.tensor_mul(ev_a[mc][:, hs], v_sb[mc][:, hs], ps_ef[:])
            # stage 4: segmented prefix sums (shift = s*PH columns inside half)
            for h in range(2):
                c0 = h * HW
                for mc in range(NCH):
                    a, b = ev_a[mc], ev_b[mc]
                    for s in (1, 2, 4, 8):
                        w_ = s * PH
                        nc.vector.tensor_add(b[:, c0 + w_:c0 + HW],
                                             a[:, c0 + w_:c0 + HW],
                                             a[:, c0:c0 + HW - w_])
                        nc.gpsimd.tensor_copy(b[:, c0:c0 + w_], a[:, c0:c0 + w_])
                        a, b = b, a
                a, b = e_sb, et
                for s in (1, 2, 4, 8):
                    w_ = s * PH
                    nc.vector.tensor_add(b[:, c0 + w_:c0 + HW],
                                         a[:, c0 + w_:c0 + HW],
                                         a[:, c0:c0 + HW - w_])
                    nc.gpsimd.tensor_copy(b[:, c0:c0 + w_], a[:, c0:c0 + w_])
                    a, b = b, a
            # stage 5: denom reciprocal via exp(-ln(cume + e0))
            for h in range(2):
                hs = HS[h]
                nc.scalar.activation(rec4[:, hs], e_sb[:, hs], AF.Ln,
                                     bias=sm4[:, 1:2])
                nc.scalar.activation(rec4[:, hs], rec4[:, hs], AF.Exp, scale=-1.0)
            # stage 6: o0 = (cumEV + init) * recip_full
            for h in range(2):
                hs = HS[h]
                for mc in range(NCH):
                    ps_rf = psA.tile([128, HW], F32, tag="t", name=f"ps_rf{h}{mc}")
                    nc.tensor.matmul(ps_rf[:], hb[:, mc, :], rec4[:, hs])
                    nc.vector.scalar_tensor_tensor(
                        o0[mc][:, hs], ev_a[mc][:, hs],
                        scal[:, S_INIT + mc:S_INIT + mc + 1], ps_rf[:],
                        op0=ALU.add, op1=ALU.mult)
            # stage 7: out-proj + residual(cls) + square
            for h in range(2):
                hs = HS[h]
                for mc in range(NCH):
                    ps_at = psA.tile([128, HW], F32, tag="t", name=f"ps_at{h}{mc}")
                    for kc in range(NCH):
                        nc.tensor.matmul(ps_at[:], wo[:, kc, mc, :], o0[kc][:, hs],
                                         start=(kc == 0), stop=(kc == NCH - 1))
                    nc.scalar.activation(pre1sq[mc][:, 0, hs], ps_at[:], AF.Identity,
                                         bias=scal[:, S_BOCLS + mc:S_BOCLS + mc + 1])
                    nc.gpsimd.tensor_mul(pre1sq[mc][:, 1, hs],
                                         pre1sq[mc][:, 0, hs], pre1sq[mc][:, 0, hs])

            def ln_stats_norm(h, srcsq, bsrc, var, wcol, bcol, dst):
                """Per-half LN over features: stats matmul -> smalls -> bcast ->
                normalize. srcsq[kc] is [128, 2, NCOL] (value | square)."""
                hs = HS[h]
                ps_st = psC.tile([1, 2, HW], F32, tag="small", name=f"st{wcol}{h}")
                for kc in range(NCH):
                    nc.tensor.matmul(ps_st[:], ones[:], srcsq[kc][:, :, hs],
                                     start=(kc == 0), stop=(kc == NCH - 1))
                m = bsrc[h][:, 0:HW]
                nc.scalar.activation(m, ps_st[:, 0, :], AF.Copy, scale=1.0 / D)
                nc.vector.tensor_mul(var[h][:], m, m)
                nc.vector.scalar_tensor_tensor(
                    var[h][:], ps_st[:, 1, :], 1.0 / D, var[h][:],
                    op0=ALU.mult, op1=ALU.subtract)
                nc.scalar.activation(var[h][:], var[h][:], AF.Ln, bias=cst[:, 0:1])
                nc.scalar.activation(bsrc[h][:, HW:2 * HW], var[h][:], AF.Exp,
                                     scale=-0.5)
                ps_bc = psA.tile([128, 256], F32, tag="t", name=f"bc{wcol}{h}")
                nc.tensor.matmul(ps_bc[:], ones1[:], bsrc[h][:])
                for kc in range(NCH):
                    nc.vector.tensor_sub(dst[kc][:, hs], srcsq[kc][:, 0, hs],
                                         ps_bc[:, 0:HW])
                    nc.vector.tensor_mul(dst[kc][:, hs], dst[kc][:, hs],
                                         ps_bc[:, HW:2 * HW])
                    nc.vector.tensor_scalar(
                        dst[kc][:, hs], dst[kc][:, hs],
                        scal[:, wcol + kc:wcol + kc + 1],
                        scal[:, bcol + kc:bcol + kc + 1],
                        op0=ALU.mult, op1=ALU.add)

            for h in range(2):
                ln_stats_norm(h, pre1sq, bsrc1, var1, S_LN1W, S_LN1B, h1)

            # ============ FF: 256 -> 2048 -> 256 (full width) ===============
            ps_h2 = [psB.tile([128, NCOL], F32, tag="acc", name=f"ps_h2_{m}")
                     for m in range(NCH)]
            for f in range(NF):
                ps_f = psA.tile([128, NCOL], F32, tag="t", name=f"ps_f{f}")
                for kc in range(NCH):
                    nc.tensor.matmul(ps_f[:], w1[:, f, kc, :], h1[kc][:],
                                     start=(kc == 0), stop=(kc == NCH - 1))
                r = rp.tile([128, NCOL], F32, tag="r", name=f"r{f}")
                if f % 2 == 0:
                    nc.scalar.activation(r[:], ps_f[:], AF.Relu,
                                         bias=b1c[:, f:f + 1])
                else:
                    nc.vector.tensor_scalar(
                        r[:], ps_f[:], b1c[:, f:f + 1], 0.0,
                        op0=ALU.add, op1=ALU.max)
                for mc in range(NCH):
                    nc.tensor.matmul(ps_h2[mc][:], w2[:, f, mc, :], r[:],
                                     start=(f == 0), stop=(f == NF - 1))

            # pre2 = ps_h2 + b2 + h1 (+ square) then LN2, per half
            for h in range(2):
                hs = HS[h]
                for mc in range(NCH):
                    nc.vector.scalar_tensor_tensor(
                        pre2sq[mc][:, 0, hs], ps_h2[mc][:, hs],
                        scal[:, S_B2 + mc:S_B2 + mc + 1],
                        h1[mc][:, hs], op0=ALU.add, op1=ALU.add)
                    nc.gpsimd.tensor_mul(pre2sq[mc][:, 1, hs],
                                         pre2sq[mc][:, 0, hs], pre2sq[mc][:, 0, hs])
            for h in range(2):
                ln_stats_norm(h, pre2sq, bsrc2, var2, S_LN2W, S_LN2B, h2)

            for mc in range(NCH):
                nc.sync.dma_start(h2o_d[mc], h2[mc][:])

            # ============ classifier (full width) ===========================
            ps_l = psC.tile([1, NCOL], F32, tag="small")
            rcl = []
            for mc in range(NCH):
                ps_c = psA.tile([128, NCOL], F32, tag="t", name=f"ps_c{mc}")
                for kc in range(NCH):
                    nc.tensor.matmul(ps_c[:], c1w[:, kc, mc, :], h2[kc][:],
                                     start=(kc == 0), stop=(kc == NCH - 1))
                rt = ap.tile([128, NCOL], F32, tag=f"rcl{mc}", name=f"rcl{mc}")
                nc.scalar.activation(rt[:], ps_c[:], AF.Relu,
                                     bias=scal[:, S_C1 + mc:S_C1 + mc + 1])
                rcl.append(rt)
            for kc in range(NCH):
                nc.tensor.matmul(ps_l[:], clf2[:, kc:kc + 1], rcl[kc][:],
                                 start=(kc == 0), stop=(kc == NCH - 1))
            p_sb = ap.tile([1, NCOL], F32, tag="p_sb")
            nc.scalar.activation(p_sb[:], ps_l[:], AF.Sigmoid, bias=cst[:, 1:2])
            nc.sync.dma_start(po_d[:], p_sb[:])

    nc.compile()
    return nc


_CACHE = {}


def _prep_host(inputs):
    f32 = np.float32
    g = lambda k: np.ascontiguousarray(np.asarray(inputs[k], f32))
    frames = g('frames')
    cls = g('cls_token')
    Wqkv, bqkv = g('in_proj_w'), g('in_proj_b')
    Wq, Wk, Wv = Wqkv[:D], Wqkv[D:2 * D], Wqkv[2 * D:]
    bq, bk, bv = bqkv[:D], bqkv[D:2 * D], bqkv[2 * D:]
    Wo, bo = g('out_proj_w'), g('out_proj_b')
    ln1w, ln1b = g('ln1_w'), g('ln1_b')
    W1, b1 = g('lin1_w'), g('lin1_b')
    W2, b2 = g('lin2_w'), g('lin2_b')
    ln2w, ln2b = g('ln2_w'), g('ln2_b')
    C1, c1 = g('clf1_w'), g('clf1_b')
    C2, c2 = g('clf2_w'), np.float32(np.asarray(inputs['clf2_b']))

    q0 = Wq @ cls + bq
    q0h = q0.reshape(H, HD)
    Wtilde = (np.einsum('hd,hdD->Dh', q0h, Wk.reshape(H, HD, D)) * SCALE).astype(f32)
    c_h = (np.einsum('hd,hd->h', q0h, bk.reshape(H, HD)) * SCALE).astype(f32)
    k_cls = (Wk @ cls + bk).reshape(H, HD)
    s0 = (np.einsum('hd,hd->h', q0h, k_cls) * SCALE).astype(f32)
    e0 = np.exp(s0)
    v_cls = Wv @ cls + bv
    init = (np.repeat(e0, HD) * v_cls).astype(f32)          # (D,)

    frames_pad = np.concatenate([frames, np.zeros((B, K, D), f32)], axis=1)
    idx = np.arange(T)[:, None] + np.arange(K)[None, :]
    Xw = frames_pad[:, idx, :].reshape(B * T, K, D)          # (128, 16, 256)

    xt_cores = []
    for c in range(NCORES):
        blk = Xw[c * PPC:(c + 1) * PPC]                      # (16, 16, 256) [p, j, d]
        # col = half*128 + j*8 + p8
        arr = blk.reshape(2, PH, K, D).transpose(3, 0, 2, 1)   # [d, h, j, p8]
        xt = np.ascontiguousarray(
            arr.reshape(NCH, 128, NCOL).transpose(1, 0, 2))
        xt_cores.append(xt)

    tchunks = lambda M: np.ascontiguousarray(                # Wt -> [128, kc, mc, 128]
        M.T.reshape(NCH, 128, NCH, 128).transpose(1, 0, 2, 3))
    wv_h = tchunks(Wv)
    wo_h = tchunks(Wo)
    c1w_h = tchunks(C1)
    w1_h = np.ascontiguousarray(                             # [128, f, kc, 128]
        W1.T.reshape(NCH, 128, NF, 128).transpose(1, 2, 0, 3))
    w2_h = np.ascontiguousarray(                             # [128, f, mc, 128]
        W2.T.reshape(NF, 128, NCH, 128).transpose(1, 0, 2, 3))

    scal = np.zeros((128, NSCAL), f32)
    pc = lambda v: v.reshape(NCH, 128).T                     # (128, 2)
    scal[:, S_BOCLS:S_BOCLS + 2] = pc(bo + cls)
    scal[:, S_BV:S_BV + 2] = pc(bv)
    scal[:, S_LN1W:S_LN1W + 2] = pc(ln1w)
    scal[:, S_LN1B:S_LN1B + 2] = pc(ln1b)
    scal[:, S_LN2W:S_LN2W + 2] = pc(ln2w)
    scal[:, S_LN2B:S_LN2B + 2] = pc(ln2b)
    scal[:, S_B2:S_B2 + 2] = pc(b2)
    scal[:, S_C1:S_C1 + 2] = pc(c1)
    scal[:, S_INIT:S_INIT + 2] = pc(init)

    sm4 = np.stack([c_h, e0], axis=1).astype(f32)            # (4, 2)
    hb_h = np.zeros((H, NCH, 128), f32)
    for h in range(H):
        for mc in range(NCH):
            for m in range(128):
                if (mc * 128 + m) // HD == h:
                    hb_h[h, mc, m] = 1.0
    b1c_h = np.ascontiguousarray(b1.reshape(NF, 128).T)
    clf2_h = np.ascontiguousarray(C2.reshape(NCH, 128).T)
    wtilde_h = np.ascontiguousarray(Wtilde.reshape(NCH, 128, H).transpose(1, 0, 2))

    common = dict(wtilde=wtilde_h, sm4=sm4, hb=hb_h, wv=wv_h, wo=wo_h,
                  c1w=c1w_h, w1=w1_h, w2=w2_h, scal=scal, b1c=b1c_h,
                  clf2=clf2_h)
    in_maps = [dict(common, xt=xt_cores[c]) for c in range(NCORES)]
    return in_maps, float(c2)


def kernel(**inputs):
    in_maps, c2 = _prep_host(inputs)
    key = 'nc'
    if key not in _CACHE or _CACHE.get('c2') != c2:
        _CACHE[key] = _build_nc(c2)
        _CACHE['c2'] = c2
    nc = _CACHE[key]

    res = run_bass_kernel_spmd(nc, in_maps, list(range(NCORES)))

    # gather: per core h2o [2, 128, 256] -> H2[b, s, L, :], po -> P[b, s, L]
    H2 = np.zeros((B * T, K, D), np.float32)
    P = np.zeros((B * T, K), np.float32)
    for c in range(NCORES):
        h2o = res.results[c]['h2o']                # (2, 128, NCOL)
        po = res.results[c]['po'].reshape(NCOL)    # (NCOL,)
        # col = half*128 + j*8 + p8
        h2r = h2o.reshape(NCH, 128, 2, K, PH)      # [kc, d, h, j, p8]
        h2r = h2r.transpose(2, 4, 3, 0, 1).reshape(PPC, K, D)
        H2[c * PPC:(c + 1) * PPC] = h2r
        P[c * PPC:(c + 1) * PPC] = (
            po.reshape(2, K, PH).transpose(0, 2, 1).reshape(PPC, K))
    H2 = H2.reshape(B, T, K, D)
    P = P.reshape(B, T, K)

    lengths = np.arange(1, K + 1)
    out = np.zeros((B, T, D), np.float32)
    for b in range(B):
        start, cnt = 0, 0
        while start < T:
            limit = min(K, T - start)
            stop = (P[b, start] >= THR) & (lengths <= limit)
            i = int(np.argmax(stop)) if stop.any() else limit - 1
            out[b, cnt] = H2[b, start, i]
            start += int(lengths[i])
            cnt += 1
    return out
